# revision 1
# baseline (speedup 1.0000x reference)
"""Trainium2 Bass kernel for nn_Geometrical_Pen (segment_reduce, memory-bound).

Computes n_pen[i] = dot(x_normals[i], y_normals[i]) / ||y_normals[0]||
for N = 16,777,216 vertices, D = 3.

Strategy (data-parallel over 8 NeuronCores):
  - Shard both [N,3] inputs along the vertex axis: 2,097,152 vertices/core.
  - Host computes the scalar 1/||y_normals[0]|| (3 floats); it is baked into
    the program as an immediate (the Bass program is built per kernel() call).
  - Per core: stream tiles of 128 partitions x F vertices ([128, 3F] f32
    contiguous HWDGE DMA loads, 3 MiB for F=2048), then on the Vector engine:
      1. tensor_mul: prod = x * y (in place)
      2. tensor_reduce over the innermost D=3 axis (AP [128, F, 3] -> X)
    then scale by 1/||y0|| on the Scalar engine and store from its HWDGE
    ring (decouples store triggers from load triggers on Sync).
  - A tail of small tiles keeps the end-of-pipeline drain short.
  - Memory-bound: 48 MiB in + 8 MiB out per core; measured ~160-165 us/core
    (~143 us pure DMA at line rate + startup/drain/barrier overhead).
"""

import sys

for _p in ("/opt/trn_rl_repo",):
    if _p not in sys.path:
        sys.path.insert(0, _p)

import numpy as np

import concourse.bacc as bacc
import concourse.mybir as mybir
from concourse.bass_utils import run_bass_kernel_spmd
from concourse.tile import TileContext


def _ensure_axon_ntff_hook():
    """Provide antenv.axon_hooks if the image's antenv lacks it.

    concourse.bass_utils unconditionally imports
    antenv.axon_hooks.get_axon_ntff_profile_hook when trace=True under
    axon; on images whose antenv predates that module the import raises
    and kills the run. Register a compatible shim backed by the same
    ctypes calls the axon boot uses, so NTFF profiling works (or
    degrades to a skipped trace when the .so lacks the symbols).
    """
    try:
        import antenv.axon_hooks  # noqa: F401

        return
    except ImportError:
        pass

    import contextlib
    import ctypes
    import types

    def _make_hook():
        so_path = "/opt/axon/libaxon_pjrt.so"
        try:
            lib = ctypes.CDLL(so_path)
        except OSError:
            return None
        if not hasattr(lib, "axon_start_nrt_profile"):
            return None
        lib.axon_start_nrt_profile.argtypes = [
            ctypes.POINTER(ctypes.c_int64),
            ctypes.c_size_t,
        ]
        lib.axon_start_nrt_profile.restype = ctypes.c_int64
        lib.axon_stop_nrt_profile.argtypes = [ctypes.c_char_p]
        lib.axon_stop_nrt_profile.restype = ctypes.c_int64

        @contextlib.contextmanager
        def _hook(output_dir, device_ids):
            import jax

            jax.devices()  # ensure the PJRT client exists in this process
            if device_ids:
                ids = (ctypes.c_int64 * len(device_ids))(*device_ids)
                rc = lib.axon_start_nrt_profile(ids, len(device_ids))
            else:
                rc = lib.axon_start_nrt_profile(None, 0)
            if rc != 0:
                raise RuntimeError(f"axon_start_nrt_profile rc={rc}")
            try:
                yield
            finally:
                n = lib.axon_stop_nrt_profile(str(output_dir).encode())
                if n < 0:
                    raise RuntimeError(f"axon_stop_nrt_profile rc={n}")
                print(f"ntff profile: {n} file(s) written to {output_dir}")

        return _hook

    holder = {"hook": _make_hook()}
    mod = types.ModuleType("antenv.axon_hooks")
    mod.get_axon_ntff_profile_hook = lambda: holder["hook"]

    def _set(h):
        holder["hook"] = h

    mod.set_axon_ntff_profile_hook = _set
    sys.modules["antenv.axon_hooks"] = mod
    try:
        import antenv

        antenv.axon_hooks = mod
    except ImportError:
        pass


_ensure_axon_ntff_hook()

N = 16777216
D = 3
NCORES = 8
P = 128                      # SBUF partitions
SHARD = N // NCORES          # 2,097,152 vertices per core

# Results of the last device run (for test harnesses to read timing info).
LAST_RESULTS = None
_NC_CACHE = {}


# Tile schedule: big tiles for DMA efficiency, then a short tail of small
# tiles so the end-of-pipeline drain (compute+store of the last-loaded
# tile, which nothing overlaps) is a few microseconds instead of ~19.
TILE_FS = [2048] * 7 + [512] * 4
assert sum(TILE_FS) * P == SHARD


def _build_nc(inv_len: float):
    # Bacc (not plain Bass): its compile pipeline legalizes instructions
    # with more than one semaphore wait, which this walrus build rejects.
    nc = bacc.Bacc(None, target_bir_lowering=False)
    x = nc.dram_tensor("x", [SHARD * D], mybir.dt.float32, kind="ExternalInput")
    y = nc.dram_tensor("y", [SHARD * D], mybir.dt.float32, kind="ExternalInput")
    out = nc.dram_tensor("out", [SHARD], mybir.dt.float32, kind="ExternalOutput")

    with TileContext(nc) as tc:
        with tc.tile_pool(name="sbuf", bufs=3) as pool:
            v0 = 0  # vertex offset within the shard
            for tf in TILE_FS:
                vt = P * tf
                xt = pool.tile([P, D * tf], mybir.dt.float32, tag="x")
                yt = pool.tile([P, D * tf], mybir.dt.float32, tag="y")
                st = pool.tile([P, tf], mybir.dt.float32, tag="s")
                xs = x[v0 * D:(v0 + vt) * D].rearrange("(p m) -> p m", p=P)
                ys = y[v0 * D:(v0 + vt) * D].rearrange("(p m) -> p m", p=P)
                nc.sync.dma_start(out=xt[:], in_=xs)
                nc.sync.dma_start(out=yt[:], in_=ys)
                # prod = x * y, in place into the x tile (DVE)
                nc.vector.tensor_mul(out=xt[:], in0=xt[:], in1=yt[:])
                # grouped sum over the innermost D=3 components (DVE)
                nc.vector.tensor_reduce(
                    out=st[:],
                    in_=xt[:].rearrange("p (f d) -> p f d", d=D),
                    axis=mybir.AxisListType.X,
                    op=mybir.AluOpType.add,
                )
                # scale by 1/||y_0|| on the otherwise-idle Scalar engine,
                # and issue the store from its HWDGE ring too, so store
                # triggers don't serialize behind load triggers on Sync.
                nc.scalar.mul(st[:], st[:], inv_len)
                od = out[v0:v0 + vt].rearrange("(p m) -> p m", p=P)
                nc.scalar.dma_start(out=od, in_=st[:])
                v0 += vt
    nc.finalize()
    return nc


def kernel(x_normals: np.ndarray, y_normals: np.ndarray) -> np.ndarray:
    global LAST_RESULTS

    x = np.ascontiguousarray(np.asarray(x_normals, dtype=np.float32))
    y = np.ascontiguousarray(np.asarray(y_normals, dtype=np.float32))
    assert x.shape == (N, D) and y.shape == (N, D)

    y0 = y[0]
    y_len = np.float32(np.sqrt(np.float32(np.sum(y0 * y0, dtype=np.float32))))
    inv_len = float(np.float32(1.0) / y_len)

    xs = x.reshape(NCORES, SHARD * D)
    ys = y.reshape(NCORES, SHARD * D)

    if inv_len not in _NC_CACHE:
        _NC_CACHE[inv_len] = _build_nc(inv_len)
    nc = _NC_CACHE[inv_len]

    in_maps = [{"x": xs[c], "y": ys[c]} for c in range(NCORES)]
    res = run_bass_kernel_spmd(nc, in_maps, core_ids=list(range(NCORES)))
    LAST_RESULTS = res

    out = np.concatenate([r["out"].reshape(-1) for r in res.results])
    return out

